# revision 3
# baseline (speedup 1.0000x reference)
"""TRN2 Bass kernel for the 4-layer encoder-with-reaches model
(nn_EncoderPreTre: B=8, S=512, D=1024, H=16 heads, NL=4 layers).

kernel(**inputs) takes the FULL inputs (src, reaches, emb_table,
qw/kw/vw/ow) and returns the full output tuple (emb, x) matching
reference.reference(). Distribution: data-parallel over the batch —
core b computes batch element b end to end (B == 8 == n_cores); the
embedding-row gather and per-batch contrib vectors are the host-side
sharding step.

Numerics: max-abs-normalized rel-err budget is 2e-2; CPU simulation
shows the error is saturated by chaotic softmax-argmax flips at the
~1e-2 level for ANY config that has at least one ~2^-11-level rounding
source in the x path (the baseline already stored P/v'/concat in fp16
for layers 0-2).  So every matmul runs with fp16 operands (1 PE
cycle/row vs fp32's 4): projections, scores, value path, out-proj.
fp16 conversions are made unbiased by pre-scaling with RNS = 1+2^-12
(RTZ converter -> round-to-nearest).  PSUM accumulation stays fp32;
the residual x stays fp32 in SBUF.

Per-core dataflow (residual transposed: x^T [1024, 512] fp32 in SBUF):
  P0: xh = fp16(x*RNS); xcb = fp16(x*contrib*RNS)  (contrib folded
      into the rhs so the OV term accumulates in the same PSUM bank as
      the out-projection).
  P1: q^T, k^T fp16 projections in [do,s] layout; v' = (x@wv)*r in
      [s,do] layout fp16.
  P2 per head: scores[q,k] fp16 MM -> row-max (DVE, negated) ->
      E=exp(s-m) fp16 with Z via accum_out -> P = E*(-c*RNS^2/Z)*mask
      in one scalar_tensor_tensor (split DVE/GPSIMD) -> P^T via 16
      DMA XBAR transposes (128x128 fp16 blocks, no PE time) ->
      M2T[dk,q] = v'^T @ P^T fp16.
  P3 per c-tile: one PSUM bank accumulates OV-part (wov16 @ xcb) +
      out-proj (wo16 @ concatT); x += bank (single DVE add).

Engine balance: PE ~100us/layer (all 1-pass MMs), DVE: row-max + 2/4
stt + q-copies + adds, ACT: exp + k/v/x copies, GPSIMD: 2/4 stt,
SP: transpose DMAs.  PSUM: psA(5) + psC(3) = 8 banks.
"""
import numpy as np

import concourse.tile as tile
from concourse import bacc, mybir
from concourse.bass_utils import run_bass_kernel_spmd

F32 = mybir.dt.float32
F32R = mybir.dt.float32r
BF16 = mybir.dt.bfloat16
FP16 = mybir.dt.float16
AX = mybir.AxisListType
OP = mybir.AluOpType
AF = mybir.ActivationFunctionType

B, S, D, H, DK, NL = 8, 512, 1024, 16, 64, 4
RNS = 1.000244140625   # 1 + 2^-12: half-ulp pre-scale so RTZ fp16 converts round-to-nearest
QC = S // 128
KC = S // 128
DC = D // 128

TRANSPOSE_MODE = "dma"   # "dma" (XBAR) or "pe" (fp16 PE transpose + copy)

TRACE = False        # test harness sets True for neuron-profile capture
LAST_RESULT = None   # BassKernelResults of the last kernel() call
_NC_CACHE = {}


def _build(n_layers=NL, n_cores=8):
    nc = bacc.Bacc("TRN2", target_bir_lowering=False, debug=False,
                   num_devices=n_cores)
    d_x0 = nc.dram_tensor("x0t", [D, S], F32, kind="ExternalInput").ap()
    d_x0h = nc.dram_tensor("x0h", [D, S], FP16, kind="ExternalInput").ap()
    d_xcb0 = nc.dram_tensor("xcb0", [D, S], FP16, kind="ExternalInput").ap()
    dw = {}
    for nm in ["wq16", "wk16", "wv16", "wov16", "wo16"]:
        dw[nm] = nc.dram_tensor(nm, [NL, D, D], FP16, kind="ExternalInput").ap()
    d_cbr = nc.dram_tensor("cbr", [128, S], F32, kind="ExternalInput").ap()
    d_negc = nc.dram_tensor("negc", [128, QC], F32, kind="ExternalInput").ap()
    d_rrn = nc.dram_tensor("rrn", [128, KC], F32, kind="ExternalInput").ap()
    d_mask = nc.dram_tensor("maskq", [QC, 128, S], FP16, kind="ExternalInput").ap()
    d_id = nc.dram_tensor("ident", [128, 128], F32, kind="ExternalInput").ap()
    d_out = nc.dram_tensor("xt", [D, S], F32, kind="ExternalOutput").ap()

    with tile.TileContext(nc) as tc:
        _emit(nc, tc, n_layers, d_x0, d_x0h, d_xcb0, dw,
              d_cbr, d_negc, d_rrn, d_mask, d_id, d_out)
    nc.compile()
    return nc


def _emit(nc, tc, n_layers, d_x0, d_x0h, d_xcb0, dw,
          d_cbr, d_negc, d_rrn, d_mask, d_id, d_out):
    ctx_pools = []

    def pool(name, bufs, space="SBUF"):
        p = tc.tile_pool(name=name, bufs=bufs, space=space)
        ctx_pools.append(p)
        return p.__enter__()

    const = pool("const", 1)
    xpool = pool("x", 1)
    hpool = pool("h", 1)          # xh / xcb fp16 tiles (1 layer live at a time)
    actp = pool("act", 1)
    wpool = pool("w", 2)          # 8 tags (per ki); bufs=2 -> next-phase prefetch
    epool = pool("E", 4)
    ppool = pool("P", 10)
    ptpool = pool("PT", 10)
    small = pool("small", 4)
    tmp8 = pool("tmp8", 1)
    psA = pool("psA", 5, "PSUM")
    psC = pool("psC", 3, "PSUM")

    cbr = const.tile([128, S], F32)
    nc.sync.dma_start(cbr[:], d_cbr)
    negc = const.tile([128, QC], F32)
    nc.sync.dma_start(negc[:], d_negc)
    rrn = const.tile([128, KC], F32, tag="rrn", name="rrn")
    nc.sync.dma_start(rrn[:], d_rrn)
    masks = []
    for t in range(QC):
        mt = const.tile([128, S], FP16, tag=f"mask{t}", name=f"mask{t}")
        nc.sync.dma_start(mt[:], d_mask[t])
        masks.append(mt)
    if TRANSPOSE_MODE == "pe":
        ident = const.tile([128, 128], F32)
        nc.sync.dma_start(ident[:], d_id)

    # residual x^T [D, S] fp32 + fp16 shadows for layer 0 (host-precast)
    xt = []
    xh0 = []
    xcb0 = []
    for c in range(DC):
        x = xpool.tile([128, S], F32, tag=f"x{c}", name=f"x{c}")
        nc.sync.dma_start(x[:], d_x0[c * 128:(c + 1) * 128, :])
        xt.append(x)
        h = hpool.tile([128, S], FP16, tag=f"xh{c}", name=f"xh0_{c}")
        nc.scalar.dma_start(h[:], d_x0h[c * 128:(c + 1) * 128, :])
        xh0.append(h)
        g = hpool.tile([128, S], FP16, tag=f"xcb{c}", name=f"xcb0_{c}")
        nc.scalar.dma_start(g[:], d_xcb0[c * 128:(c + 1) * 128, :])
        xcb0.append(g)

    for l in range(n_layers):
        last = l == n_layers - 1

        if l == 0:
            xh, xcb = xh0, xcb0
        else:
            xh, xcb = [], []
            for c in range(DC):
                hc = hpool.tile([128, S], FP16, tag=f"xh{c}", name=f"xh{c}_{l}")
                nc.scalar.activation(hc[:], xt[c][:], AF.Copy, scale=RNS)
                xh.append(hc)
                gc = hpool.tile([128, S], FP16, tag=f"xcb{c}", name=f"xcb{c}_{l}")
                nc.vector.tensor_tensor(gc[:], xt[c][:], cbr[:], op=OP.mult)
                xcb.append(gc)

        def load_w(nm):
            ws = []
            for ki in range(DC):
                w = wpool.tile([128, D], FP16, tag=f"w{ki}", name=f"{nm}{ki}_{l}")
                eng = nc.sync if ki % 2 == 0 else nc.scalar
                eng.dma_start(w[:], dw[nm][l, ki * 128:(ki + 1) * 128, :])
                ws.append(w)
            return ws

        # q^T / k^T projections, fp16 [do, s]
        wq = load_w("wq16")
        qt = []
        for c in range(DC):
            p = psA.tile([128, S], F32, tag="psA", name=f"pq{c}_{l}")
            sl = slice(c * 128, (c + 1) * 128)
            for ki in range(DC):
                nc.tensor.matmul(p[:], wq[ki][:, sl], xh[ki][:],
                                 start=(ki == 0), stop=(ki == DC - 1),
                                 skip_group_check=True)
            o = actp.tile([128, S], FP16, tag=f"qt{c}", name=f"qt{c}_{l}")
            nc.vector.tensor_scalar(o[:], p[:], RNS, None, op0=OP.mult)
            qt.append(o)
        wk = load_w("wk16")
        kt = []
        for c in range(DC):
            p = psA.tile([128, S], F32, tag="psA", name=f"pk{c}_{l}")
            sl = slice(c * 128, (c + 1) * 128)
            for ki in range(DC):
                nc.tensor.matmul(p[:], wk[ki][:, sl], xh[ki][:],
                                 start=(ki == 0), stop=(ki == DC - 1),
                                 skip_group_check=True)
            o = actp.tile([128, S], FP16, tag=f"kt{c}", name=f"kt{c}_{l}")
            nc.scalar.activation(o[:], p[:], AF.Copy, scale=RNS)
            kt.append(o)

        # v' = (x @ wv) * r in [s, dv] layout fp16 (rrn carries r*RNS)
        wv = load_w("wv16")
        vp = []
        for sc in range(KC):
            vtile = actp.tile([128, D], FP16, tag=f"vp{sc}", name=f"vp{sc}_{l}")
            ssl = slice(sc * 128, (sc + 1) * 128)
            for half in range(2):
                hsl = slice(half * 512, (half + 1) * 512)
                p = psA.tile([128, S], F32, tag="psA", name=f"pv{sc}{half}_{l}")
                for ki in range(DC):
                    nc.tensor.matmul(p[:], xh[ki][:, ssl], wv[ki][:, hsl],
                                     start=(ki == 0), stop=(ki == DC - 1),
                                     skip_group_check=True)
                nc.scalar.activation(vtile[:, hsl], p[:], AF.Copy,
                                     scale=rrn[:, sc:sc + 1])
            vp.append(vtile)

        wov = load_w("wov16")

        concatT = [actp.tile([128, S], FP16, tag=f"cc{c}", name=f"cc{c}_{l}")
                   for c in range(DC)]
        for h in range(H):
            hp = h // 2
            hb = (h % 2) * 64
            qsl = qt[hp][hb:hb + 64, :]
            ksl = kt[hp][hb:hb + 64, :]

            negm = small.tile([128, QC], F32, tag="negm", name=f"negm{h}_{l}")
            zst = small.tile([128, QC], F32, tag="zst", name=f"zst{h}_{l}")
            sct = small.tile([128, QC], F32, tag="scl", name=f"scl{h}_{l}")
            PTs = [ptpool.tile([128, S], FP16, tag="PT", name=f"pt{h}{kc}_{l}")
                   for kc in range(KC)]
            Ps = []
            for t in range(QC):
                ps = psA.tile([128, S], F32, tag="psA", name=f"sc{h}{t}_{l}")
                nc.tensor.matmul(ps[:], qsl[:, t * 128:(t + 1) * 128], ksl,
                                 start=True, stop=True)
                nc.vector.tensor_reduce(
                    negm[:, t:t + 1], ps[:], axis=AX.X, op=OP.max, negate=True)
                e = epool.tile([128, S], FP16, tag="E", name=f"e{h}{t}_{l}")
                nc.scalar.activation(e[:], ps[:], AF.Exp,
                                     bias=negm[:, t:t + 1], scale=1.0,
                                     accum_out=zst[:, t:t + 1])
                nc.vector.reciprocal(sct[:, t:t + 1], zst[:, t:t + 1])
                nc.vector.tensor_tensor(
                    sct[:, t:t + 1], sct[:, t:t + 1], negc[:, t:t + 1],
                    op=OP.mult)
                pp = ppool.tile([128, S], FP16, tag="P", name=f"p{h}{t}_{l}")
                nc.vector.scalar_tensor_tensor(
                    pp[:], e[:], sct[:, t:t + 1], masks[t][:],
                    op0=OP.mult, op1=OP.mult)
                Ps.append(pp)
                if TRANSPOSE_MODE == "dma":
                    for kc in range(KC):
                        nc.sync.dma_start(
                            PTs[kc][:, t * 128:(t + 1) * 128],
                            pp[:, kc * 128:(kc + 1) * 128], transpose=True)
            if TRANSPOSE_MODE == "pe":
                for kc in range(KC):
                    tp = psC.tile([128, S], F32, tag="psC", name=f"tp{h}{kc}_{l}")
                    for t in range(QC):
                        nc.tensor.matmul(
                            tp[:, t * 128:(t + 1) * 128],
                            Ps[t][:, kc * 128:(kc + 1) * 128], ident[:],
                            is_transpose=True, start=(t == 0), stop=(t == QC - 1),
                            skip_group_check=True)
                    if kc % 2 == 0:
                        nc.vector.tensor_copy(PTs[kc][:], tp[:])
                    else:
                        nc.scalar.copy(PTs[kc][:], tp[:])

            m2 = psC.tile([128, S], F32, tag="psC", name=f"m2{h}_{l}")
            for kc in range(KC):
                nc.tensor.matmul(
                    m2[0:64, :], vp[kc][:, h * 64:h * 64 + 64],
                    PTs[kc][:], start=(kc == 0), stop=(kc == KC - 1))
            nc.scalar.activation(concatT[hp][hb:hb + 64, :], m2[0:64, :],
                                 AF.Copy, scale=RNS)

        # out-proj + OV term in one PSUM accumulation per c-tile
        wo = load_w("wo16")
        for c in range(DC):
            pw = psA.tile([128, S], F32, tag="psA", name=f"pow{c}_{l}")
            sl = slice(c * 128, (c + 1) * 128)
            for ki in range(DC):
                nc.tensor.matmul(pw[:], wov[ki][:, sl], xcb[ki][:],
                                 start=(ki == 0), stop=False,
                                 skip_group_check=True)
            for ki in range(DC):
                nc.tensor.matmul(pw[:], wo[ki][:, sl], concatT[ki][:],
                                 start=False, stop=(ki == DC - 1),
                                 skip_group_check=True)
            if last:
                xf = tmp8.tile([128, S], F32, tag="xf", name=f"xf{c}_{l}",
                               bufs=2)
                nc.vector.tensor_tensor(xf[:], xt[c][:], pw[:], op=OP.add)
                nc.sync.dma_start(d_out[c * 128:(c + 1) * 128, :], xf[:])
            else:
                nc.vector.tensor_tensor(xt[c][:], xt[c][:], pw[:], op=OP.add)

    for p in reversed(ctx_pools):
        p.__exit__(None, None, None)


# ---------------- host side ----------------

def _host_prep(src, reaches, emb_table, qw, kw, vw, ow):
    src = np.asarray(src)
    reaches = np.asarray(reaches, dtype=np.float32)
    emb_table = np.asarray(emb_table, dtype=np.float32)
    emb = emb_table[src]
    rs = reaches.sum(-1, keepdims=True)
    contrib = ((rs - reaches) / (rs + 1e-9) * (1.0 - reaches) * 100.0
               ).astype(np.float32)

    qw = np.asarray(qw, np.float32); kw = np.asarray(kw, np.float32)
    vw = np.asarray(vw, np.float32); ow = np.asarray(ow, np.float32)
    wq16 = np.ascontiguousarray(
        np.transpose(qw, (0, 2, 1)) * np.float32(0.125)).astype(np.float16)
    wk16 = np.ascontiguousarray(np.transpose(kw, (0, 2, 1))).astype(np.float16)
    wv16 = np.ascontiguousarray(np.transpose(vw, (0, 2, 1))).astype(np.float16)
    wo16 = np.ascontiguousarray(np.transpose(ow, (0, 2, 1))).astype(np.float16)
    wov16 = np.stack([
        np.ascontiguousarray(
            (ow[l].astype(np.float64) @ vw[l].astype(np.float64)).T
        ).astype(np.float16)
        for l in range(NL)])

    maskq = np.ones((QC, 128, S), np.float16)
    idx = np.arange(128)
    diagval = np.float32(1.0) - np.float32(0.999999)
    for t in range(QC):
        maskq[t, idx, t * 128 + idx] = np.float16(diagval)
    ident = np.eye(128, dtype=np.float32)

    rns = np.float32(RNS)
    shared = dict(wq16=wq16, wk16=wk16, wv16=wv16, wov16=wov16, wo16=wo16,
                  maskq=maskq, ident=ident)
    in_maps = []
    for b in range(B):
        x0t = np.ascontiguousarray(emb[b].T)
        in_maps.append(dict(
            shared,
            x0t=x0t,
            x0h=(x0t * rns).astype(np.float16),
            xcb0=(x0t * (contrib[b][None, :] * rns)).astype(np.float16),
            cbr=np.ascontiguousarray(
                np.broadcast_to(contrib[b][None, :] * rns, (128, S))),
            negc=np.ascontiguousarray(
                -contrib[b].reshape(QC, 128).T * rns * rns),
            rrn=np.ascontiguousarray(
                reaches[b].reshape(KC, 128).T * rns),
        ))
    return emb, in_maps


def kernel(src, reaches, emb_table, qw, kw, vw, ow):
    global LAST_RESULT
    if "nc" not in _NC_CACHE:
        _NC_CACHE["nc"] = _build(n_layers=NL, n_cores=B)
    nc = _NC_CACHE["nc"]
    emb, in_maps = _host_prep(src, reaches, emb_table, qw, kw, vw, ow)
    res = run_bass_kernel_spmd(nc, in_maps, core_ids=list(range(B)),
                               trace=TRACE)
    LAST_RESULT = res
    x = np.stack([r["xt"].T for r in res.results]).astype(np.float32)
    return emb, x


# revision 17
# speedup vs baseline: 2.1960x; 2.1960x over previous
"""TRN2 Bass kernel for the 4-layer encoder-with-reaches model
(nn_EncoderPreTre: B=8, S=512, D=1024, H=16 heads, NL=4 layers).

kernel(**inputs) takes the FULL inputs (src, reaches, emb_table,
qw/kw/vw/ow) and returns the full output tuple (emb, x) matching
reference.reference(). Distribution: data-parallel over the batch —
core b computes batch element b end to end (B == 8 == n_cores); the
embedding-row gather and per-batch contrib vectors are the host-side
sharding step.

Numerics: max-abs-normalized rel-err budget is 2e-2; CPU simulation
shows the error is saturated by chaotic softmax-argmax flips at the
~1e-2 level for ANY config that has at least one ~2^-11-level rounding
source in the x path (the baseline already stored P/v'/concat in fp16
for layers 0-2).  So every matmul runs with fp16 operands (1 PE
cycle/row vs fp32's 4): projections, scores, value path, out-proj.
fp16 conversions are made unbiased by pre-scaling with RNS = 1+2^-12
(RTZ converter -> round-to-nearest).  PSUM accumulation stays fp32;
the residual x stays fp32 in SBUF.

Per-core dataflow (residual transposed: x^T [1024, 512] fp32 in SBUF):
  P0: xh = fp16(x*RNS); xcb = fp16(x*contrib*RNS)  (contrib folded
      into the rhs so the OV term accumulates in the same PSUM bank as
      the out-projection).
  P1: q^T, k^T fp16 projections in [do,s] layout; v' = (x@wv)*r in
      [s,do] layout fp16.
  P2 per head: scores[q,k] fp16 MM -> row-max (DVE, negated) ->
      E=exp(s-m) fp16 with Z via accum_out -> P = E*(-c*RNS^2/Z)*mask
      in one scalar_tensor_tensor (split DVE/GPSIMD) -> P^T via 16
      DMA XBAR transposes (128x128 fp16 blocks, no PE time) ->
      M2T[dk,q] = v'^T @ P^T fp16.
  P3 per c-tile: one PSUM bank accumulates OV-part (wov16 @ xcb) +
      out-proj (wo16 @ concatT); x += bank (single DVE add).

Engine balance: PE ~100us/layer (all 1-pass MMs), DVE: row-max + 2/4
stt + q-copies + adds, ACT: exp + k/v/x copies, GPSIMD: 2/4 stt,
SP: transpose DMAs.  PSUM: psA(5) + psC(3) = 8 banks.
"""
import numpy as np

import concourse.tile as tile
from concourse import bacc, mybir
from concourse.bass_utils import run_bass_kernel_spmd

F32 = mybir.dt.float32
F32R = mybir.dt.float32r
BF16 = mybir.dt.bfloat16
FP16 = mybir.dt.float16
AX = mybir.AxisListType
OP = mybir.AluOpType
AF = mybir.ActivationFunctionType

B, S, D, H, DK, NL = 8, 512, 1024, 16, 64, 4
RNS = 1.000244140625   # 1 + 2^-12: half-ulp pre-scale so RTZ fp16 converts round-to-nearest
QC = S // 128
KC = S // 128
DC = D // 128

TRANSPOSE_MODE = "pe"    # "dma" (XBAR) or "pe" (fp16 PE transpose + copy)
# DMA transpose measured: ~1.2us per 128x128 call, serialized on the one
# HWDGE queue -> 1254us for 1024 calls. PE fp16 transpose is 1 cyc/row.

TRACE = False        # test harness sets True for neuron-profile capture
LAST_RESULT = None   # BassKernelResults of the last kernel() call
_NC_CACHE = {}


def _build(n_layers=NL, n_cores=8):
    nc = bacc.Bacc("TRN2", target_bir_lowering=False, debug=False,
                   num_devices=n_cores)
    d_x0 = nc.dram_tensor("x0t", [D, S], F32, kind="ExternalInput").ap()
    d_x0h = nc.dram_tensor("x0h", [D, S], FP16, kind="ExternalInput").ap()
    d_xcb0 = nc.dram_tensor("xcb0", [D, S], FP16, kind="ExternalInput").ap()
    dw = {}
    for nm in ["wq16", "wk16", "wv16", "wov16", "wo16"]:
        dw[nm] = nc.dram_tensor(nm, [NL, D, D], FP16, kind="ExternalInput").ap()
    d_cbr = nc.dram_tensor("cbr", [128, S], F32, kind="ExternalInput").ap()
    d_negc = nc.dram_tensor("negc", [128, QC], F32, kind="ExternalInput").ap()
    d_negcl = nc.dram_tensor("negcl", [128, QC], F32, kind="ExternalInput").ap()
    d_rrn = nc.dram_tensor("rrn", [128, KC], F32, kind="ExternalInput").ap()
    d_rr = nc.dram_tensor("rr", [128, KC], F32, kind="ExternalInput").ap()
    d_mask = nc.dram_tensor("maskq", [QC, 128, S], FP16, kind="ExternalInput").ap()
    d_id = nc.dram_tensor("ident", [128, 128], FP16, kind="ExternalInput").ap()
    d_idr = nc.dram_tensor("identr", [128, 128], F32R, kind="ExternalInput").ap()
    dw["wo32"] = nc.dram_tensor("wo32", [D, D], F32R, kind="ExternalInput").ap()
    d_out = nc.dram_tensor("xt", [D, S], F32, kind="ExternalOutput").ap()

    with tile.TileContext(nc) as tc:
        _emit(nc, tc, n_layers, d_x0, d_x0h, d_xcb0, dw,
              d_cbr, (d_negc, d_negcl), (d_rrn, d_rr), d_mask, (d_id, d_idr),
              d_out)
    nc.compile()
    return nc


def _emit(nc, tc, n_layers, d_x0, d_x0h, d_xcb0, dw,
          d_cbr, d_negc2, d_rrn2, d_mask, d_id2, d_out):
    d_negc, d_negcl = d_negc2
    d_rrn, d_rr = d_rrn2
    d_id, d_idr = d_id2
    ctx_pools = []

    def pool(name, bufs, space="SBUF"):
        p = tc.tile_pool(name=name, bufs=bufs, space=space)
        ctx_pools.append(p)
        return p.__enter__()

    const = pool("const", 1)
    xpool = pool("x", 1)
    hpool = pool("h", 1)          # xh / xcb fp16 tiles (1 layer live at a time)
    actp = pool("act", 1)
    wpool = pool("w", 2)          # 8 tags (per ki); bufs=2 -> next-phase prefetch
    epool = pool("E", 4)
    ppool = pool("P", 10)
    ptpool = pool("PT", 10)
    small = pool("small", 4)
    tmp8 = pool("tmp8", 1)
    psA = pool("psA", 4, "PSUM")
    psC = pool("psC", 4, "PSUM")

    cbr = const.tile([128, S], F32)
    nc.sync.dma_start(cbr[:], d_cbr)
    negc = const.tile([128, QC], F32)
    nc.sync.dma_start(negc[:], d_negc)
    negcl = const.tile([128, QC], F32, tag="negcl", name="negcl")
    nc.sync.dma_start(negcl[:], d_negcl)
    rrn = const.tile([128, KC], F32, tag="rrn", name="rrn")
    nc.sync.dma_start(rrn[:], d_rrn)
    rr = const.tile([128, KC], F32, tag="rr", name="rr")
    nc.sync.dma_start(rr[:], d_rr)
    masks = []
    for t in range(QC):
        mt = const.tile([128, S], FP16, tag=f"mask{t}", name=f"mask{t}")
        nc.sync.dma_start(mt[:], d_mask[t])
        masks.append(mt)
    ident = const.tile([128, 128], FP16)
    nc.sync.dma_start(ident[:], d_id)
    identr = const.tile([128, 128], F32R, tag="identr", name="identr")
    nc.sync.dma_start(identr[:], d_idr)
    wo32 = []
    for ki in range(DC):
        w = const.tile([128, D], F32R, tag=f"wo32_{ki}", name=f"wo32_{ki}")
        eng = nc.sync if ki % 2 == 0 else nc.scalar
        eng.dma_start(w[:], dw["wo32"][ki * 128:(ki + 1) * 128, :])
        wo32.append(w)

    # residual x^T [D, S] fp32 + fp16 shadows for layer 0 (host-precast)
    xt = []
    xh0 = []
    xcb0 = []
    for c in range(DC):
        x = xpool.tile([128, S], F32, tag=f"x{c}", name=f"x{c}")
        nc.sync.dma_start(x[:], d_x0[c * 128:(c + 1) * 128, :])
        xt.append(x)
        h = hpool.tile([128, S], FP16, tag=f"xh{c}", name=f"xh0_{c}")
        nc.scalar.dma_start(h[:], d_x0h[c * 128:(c + 1) * 128, :])
        xh0.append(h)
        g = hpool.tile([128, S], FP16, tag=f"xcb{c}", name=f"xcb0_{c}")
        nc.scalar.dma_start(g[:], d_xcb0[c * 128:(c + 1) * 128, :])
        xcb0.append(g)

    for l in range(n_layers):
        last = l == n_layers - 1

        if l == 0:
            xh, xcb = xh0, xcb0
        else:
            xh, xcb = [], []
            for c in range(DC):
                hc = hpool.tile([128, S], FP16, tag=f"xh{c}", name=f"xh{c}_{l}")
                nc.scalar.activation(hc[:], xt[c][:], AF.Copy, scale=RNS)
                xh.append(hc)
                gc = hpool.tile([128, S], FP16, tag=f"xcb{c}", name=f"xcb{c}_{l}")
                nc.vector.tensor_tensor(gc[:], xt[c][:], cbr[:], op=OP.mult)
                xcb.append(gc)

        def load_w(nm):
            ws = []
            for ki in range(DC):
                w = wpool.tile([128, D], FP16, tag=f"w{ki}", name=f"{nm}{ki}_{l}")
                eng = nc.sync if ki % 2 == 0 else nc.scalar
                eng.dma_start(w[:], dw[nm][l, ki * 128:(ki + 1) * 128, :])
                ws.append(w)
            return ws

        # q^T / k^T projections, fp16 [do, s]
        wq = load_w("wq16")
        qt = []
        for c in range(DC):
            p = psA.tile([128, S], F32, tag="psA", name=f"pq{c}_{l}")
            sl = slice(c * 128, (c + 1) * 128)
            for ki in range(DC):
                nc.tensor.matmul(p[:], wq[ki][:, sl], xh[ki][:],
                                 start=(ki == 0), stop=(ki == DC - 1),
                                 skip_group_check=True)
            o = actp.tile([128, S], FP16, tag=f"qt{c}", name=f"qt{c}_{l}")
            nc.vector.tensor_scalar(o[:], p[:], RNS, None, op0=OP.mult)
            qt.append(o)
        wk = load_w("wk16")
        kt = []
        for c in range(DC):
            p = psA.tile([128, S], F32, tag="psA", name=f"pk{c}_{l}")
            sl = slice(c * 128, (c + 1) * 128)
            for ki in range(DC):
                nc.tensor.matmul(p[:], wk[ki][:, sl], xh[ki][:],
                                 start=(ki == 0), stop=(ki == DC - 1),
                                 skip_group_check=True)
            o = actp.tile([128, S], FP16, tag=f"kt{c}", name=f"kt{c}_{l}")
            nc.scalar.activation(o[:], p[:], AF.Copy, scale=RNS)
            kt.append(o)

        # v' = (x @ wv) * r in [s, dv] layout (rrn carries r*RNS for fp16;
        # layer 3 uses f32r with plain r — concat/P range exceeds fp16)
        vdt = F32R if last else FP16
        vscale = rr if last else rrn
        wv = load_w("wv16")
        vp = []
        for sc in range(KC):
            vtile = actp.tile([128, D], vdt, tag=f"vp{sc}", name=f"vp{sc}_{l}")
            ssl = slice(sc * 128, (sc + 1) * 128)
            for half in range(2):
                hsl = slice(half * 512, (half + 1) * 512)
                p = psA.tile([128, S], F32, tag="psA", name=f"pv{sc}{half}_{l}")
                for ki in range(DC):
                    nc.tensor.matmul(p[:], xh[ki][:, ssl], wv[ki][:, hsl],
                                     start=(ki == 0), stop=(ki == DC - 1),
                                     skip_group_check=True)
                nc.scalar.activation(vtile[:, hsl], p[:], AF.Copy,
                                     scale=vscale[:, sc:sc + 1])
            vp.append(vtile)

        wov = load_w("wov16")

        # layer 3 concat reaches ~5e5 — beyond fp16 range; f32r keeps range
        # at the same 1-cycle/row matmul rate
        cdt = F32R if last else FP16
        pdt = F32R if last else FP16
        id_l = identr if last else ident
        sc_neg = negcl if last else negc
        skipmax = l < 2          # |scores| <= ~2 at layers 0-1: exp is safe
        concatT = [actp.tile([128, S], cdt, tag=f"cc{c}", name=f"cc{c}_{l}")
                   for c in range(DC)]
        # heads processed in pairs: head 2j on partitions 0-63 of qt/kt[j],
        # head 2j+1 on 64-127.  Scores run as two row-tiles (T0/T8) and M2
        # as two column-tiles (T0/T1) so the pair shares the PE array.
        for j in range(H // 2):
            qk = []
            for hb in (0, 64):
                qk.append((qt[j][hb:hb + 64, :], kt[j][hb:hb + 64, :]))
            negm = small.tile([128, 2 * QC], F32, tag="negm", name=f"nm{j}_{l}")
            zst = small.tile([128, 2 * QC], F32, tag="zst", name=f"zs{j}_{l}")
            sct = small.tile([128, 2 * QC], F32, tag="scl", name=f"sl{j}_{l}")
            PTs = [[ptpool.tile([128, S], pdt, tag="PT",
                                name=f"pt{j}{hi}{kc}_{l}")
                    for kc in range(KC)] for hi in range(2)]
            Ps = [[], []]
            for t in range(QC):
                for hi in range(2):
                    qsl, ksl = qk[hi]
                    ti = 2 * t + hi
                    ps = psA.tile([128, S], F32, tag="psA",
                                  name=f"sc{j}{ti}_{l}")
                    nc.tensor.matmul(ps[:], qsl[:, t * 128:(t + 1) * 128],
                                     ksl, start=True, stop=True)
                    e = epool.tile([128, S], F32, tag="E", name=f"e{j}{ti}_{l}")
                    if skipmax:
                        nc.scalar.activation(e[:], ps[:], AF.Exp,
                                             bias=0.0, scale=1.0,
                                             accum_out=zst[:, ti:ti + 1])
                    else:
                        nc.vector.tensor_reduce(
                            negm[:, ti:ti + 1], ps[:], axis=AX.X, op=OP.max,
                            negate=True)
                        nc.scalar.activation(e[:], ps[:], AF.Exp,
                                             bias=negm[:, ti:ti + 1], scale=1.0,
                                             accum_out=zst[:, ti:ti + 1])
                    nc.vector.reciprocal(sct[:, ti:ti + 1], zst[:, ti:ti + 1])
                    nc.vector.tensor_tensor(
                        sct[:, ti:ti + 1], sct[:, ti:ti + 1],
                        sc_neg[:, t:t + 1], op=OP.mult)
                    pp = ppool.tile([128, S], pdt, tag="P",
                                    name=f"p{j}{ti}_{l}")
                    nc.vector.scalar_tensor_tensor(
                        pp[:], e[:], sct[:, ti:ti + 1], masks[t][:],
                        op0=OP.mult, op1=OP.mult)
                    Ps[hi].append(pp)
            for hi in range(2):
                for kc in range(KC):
                    tp = psC.tile([128, S], pdt, tag="psC",
                                  name=f"tp{j}{hi}{kc}_{l}")
                    for t in range(QC):
                        nc.tensor.matmul(
                            tp[:, t * 128:(t + 1) * 128],
                            Ps[hi][t][:, kc * 128:(kc + 1) * 128], id_l[:],
                            is_transpose=True, start=(t == 0),
                            stop=(t == QC - 1), skip_group_check=True)
                    # P was already 16/32-bit rounded: PSUM->SBUF copy exact
                    if kc % 2 == 0:
                        nc.vector.tensor_copy(PTs[hi][kc][:], tp[:])
                    else:
                        nc.scalar.copy(PTs[hi][kc][:], tp[:])

            # (col-tiled dst partitions 64-127 fail the s3d3 ISA check, so
            # each head's M2 targets partitions 0-63 of its own bank)
            for hi in range(2):
                h = 2 * j + hi
                m2 = psC.tile([128, S], F32, tag="psC", name=f"m2{j}{hi}_{l}")
                for kc in range(KC):
                    nc.tensor.matmul(
                        m2[0:64, :], vp[kc][:, h * 64:h * 64 + 64],
                        PTs[hi][kc][:], start=(kc == 0), stop=(kc == KC - 1),
                        skip_group_check=True)
                if last:
                    nc.scalar.copy(concatT[j][64 * hi:64 * hi + 64, :],
                                   m2[0:64, :])
                else:
                    nc.scalar.activation(concatT[j][64 * hi:64 * hi + 64, :],
                                         m2[0:64, :], AF.Copy, scale=RNS)

        # out-proj + OV term in one PSUM accumulation per c-tile
        wo = wo32 if last else load_w("wo16")
        for c in range(DC):
            pw = psA.tile([128, S], F32, tag="psA", name=f"pow{c}_{l}")
            sl = slice(c * 128, (c + 1) * 128)
            for ki in range(DC):
                nc.tensor.matmul(pw[:], wov[ki][:, sl], xcb[ki][:],
                                 start=(ki == 0), stop=False,
                                 skip_group_check=True)
            for ki in range(DC):
                nc.tensor.matmul(pw[:], wo[ki][:, sl], concatT[ki][:],
                                 start=False, stop=(ki == DC - 1),
                                 skip_group_check=True)
            if last:
                xf = tmp8.tile([128, S], F32, tag="xf", name=f"xf{c}_{l}",
                               bufs=2)
                nc.vector.tensor_tensor(xf[:], xt[c][:], pw[:], op=OP.add)
                nc.sync.dma_start(d_out[c * 128:(c + 1) * 128, :], xf[:])
            else:
                nc.vector.tensor_tensor(xt[c][:], xt[c][:], pw[:], op=OP.add)

    for p in reversed(ctx_pools):
        p.__exit__(None, None, None)


# ---------------- host side ----------------

def _host_prep(src, reaches, emb_table, qw, kw, vw, ow):
    src = np.asarray(src)
    reaches = np.asarray(reaches, dtype=np.float32)
    emb_table = np.asarray(emb_table, dtype=np.float32)
    emb = emb_table[src]
    rs = reaches.sum(-1, keepdims=True)
    contrib = ((rs - reaches) / (rs + 1e-9) * (1.0 - reaches) * 100.0
               ).astype(np.float32)

    qw = np.asarray(qw, np.float32); kw = np.asarray(kw, np.float32)
    vw = np.asarray(vw, np.float32); ow = np.asarray(ow, np.float32)
    wq16 = np.ascontiguousarray(
        np.transpose(qw, (0, 2, 1)) * np.float32(0.125)).astype(np.float16)
    wk16 = np.ascontiguousarray(np.transpose(kw, (0, 2, 1))).astype(np.float16)
    wv16 = np.ascontiguousarray(np.transpose(vw, (0, 2, 1))).astype(np.float16)
    wo16 = np.ascontiguousarray(np.transpose(ow, (0, 2, 1))).astype(np.float16)
    # xcb = x*contrib/64 (fp16 range: |x3*c| ~ 7e5 > fp16 max); x64 into wov
    wov16 = np.stack([
        np.ascontiguousarray(
            (ow[l].astype(np.float64) @ vw[l].astype(np.float64)).T * 64.0
        ).astype(np.float16)
        for l in range(NL)])
    wo32 = np.ascontiguousarray(np.transpose(ow[NL - 1]))

    maskq = np.ones((QC, 128, S), np.float16)
    idx = np.arange(128)
    diagval = np.float32(1.0) - np.float32(0.999999)
    for t in range(QC):
        maskq[t, idx, t * 128 + idx] = np.float16(diagval)
    ident = np.eye(128, dtype=np.float16)
    identr = np.eye(128, dtype=np.float32)

    rns = np.float32(RNS)
    cscale = contrib * (rns / np.float32(64.0))
    shared = dict(wq16=wq16, wk16=wk16, wv16=wv16, wov16=wov16, wo16=wo16,
                  wo32=wo32, maskq=maskq, ident=ident, identr=identr)
    in_maps = []
    for b in range(B):
        x0t = np.ascontiguousarray(emb[b].T)
        in_maps.append(dict(
            shared,
            x0t=x0t,
            x0h=(x0t * rns).astype(np.float16),
            xcb0=(x0t * cscale[b][None, :]).astype(np.float16),
            cbr=np.ascontiguousarray(
                np.broadcast_to(cscale[b][None, :], (128, S))),
            negc=np.ascontiguousarray(
                -contrib[b].reshape(QC, 128).T * rns),
            negcl=np.ascontiguousarray(
                -contrib[b].reshape(QC, 128).T),
            rrn=np.ascontiguousarray(
                reaches[b].reshape(KC, 128).T * rns),
            rr=np.ascontiguousarray(
                reaches[b].reshape(KC, 128).T),
        ))
    return emb, in_maps


def kernel(src, reaches, emb_table, qw, kw, vw, ow):
    global LAST_RESULT
    if "nc" not in _NC_CACHE:
        _NC_CACHE["nc"] = _build(n_layers=NL, n_cores=B)
    nc = _NC_CACHE["nc"]
    emb, in_maps = _host_prep(src, reaches, emb_table, qw, kw, vw, ow)
    res = run_bass_kernel_spmd(nc, in_maps, core_ids=list(range(B)),
                               trace=TRACE)
    LAST_RESULT = res
    x = np.stack([r["xt"].T for r in res.results]).astype(np.float32)
    return emb, x
